# revision 83
# baseline (speedup 1.0000x reference)
"""Trainium2 Bass kernel for factorized spatial attention (nn_Attention_50379966382361).

Reference computation (per batch b, frame f):
    qkv = x @ Wqkv.T ; split into q,k,v heads (8 heads, hd=64)
    attn = softmax(q @ k.T * hd**-0.5) over spatial tokens (n=784) within the frame
    out  = attn @ v ; merge heads ; y = out @ Wproj.T + bproj

Sharding: data-parallel over the 32 (b, f) frames -> 4 frames per core, weights
replicated.

v2 design (head-serial attention, ACT-exp bound):
  - scoresT[j, i] per (head, j-tile) via fp8e4 DoubleRow matmuls (0.5 PE
    cycles/row): q,k are cast PSUM->SBUF into [128, 2, 784] fp8 tiles whose
    second k-tile half is zeroed once at startup (zeros contribute nothing to
    the DR contraction, so no layout interleave is needed).  hd^-0.5 is folded
    into Wq on the host.
  - one exp per (head, j-tile) on ACT ([112, 784], PSUM->SBUF bf16).  ACT is
    the bottleneck engine (~47us/frame); everything else is kept off ACT.
  - attn@v reoriented: out[i-tile, 65] = e[j,i-slice].T @ [v | 1] with bf16
    operands -> 65-row matmuls instead of 784-row, accumulated over j-tiles
    into one PSUM bank per head (col 64 of each 65-block = softmax denom).
    Any PSUM bank holding multiple accumulation groups is zeroed by a single
    start=True K=1 matmul and every real matmul runs start=False with a RAW
    dep on it: the PSUM zero-region is 2KB-granular and the Tile scheduler
    freely interleaves groups, so per-group start=True flags corrupt
    neighbouring groups mid-accumulation (observed on hardware).
  - normalize: DVE reciprocal of the denominators + one tensor_tensor mult per
    head with a stride-0-broadcast multiplier -> attnout [i, d] bf16.
  - attnout -> channel-major via plain bf16 matmuls against a host-supplied
    identity (the DMA XBAR transpose faults the core on this runtime), then
    the projection runs on bf16 with rectangular token chunks; bias on DVE
    (GPSIMD cannot access PSUM here), store via sync DMA.
  - cross-frame software pipeline: QKV matmuls of frame f+1 and the projection
    of frame f-1 are emitted as "fill" items into the PE-idle slots of frame
    f's ACT-bound attention loop.
"""

import os

import numpy as np

import concourse.bass as bass
import concourse.mybir as mybir
import concourse.tile as tile

B, F, N, VD, D, H = 2, 16, 784, 512, 512, 8
HD = D // H                      # 64
NCORES = 8
FPC = (B * F) // NCORES          # frames per core = 4
JT = 112                         # j-tile (7 * 112 = 784)
NJ = N // JT                     # 7
IT0 = (0, 128, 256, 384, 512, 640, 656)  # i-tile starts (width 128, last overlaps)
PIPE = 3                         # attn@v trails exp by this many j-tiles
CHUNKS = ((0, 512), (512, 272))  # free-dim chunks of 784 (PSUM bank = 512 fp32)
FP32 = mybir.dt.float32
F32R = mybir.dt.float32r
BF16 = mybir.dt.bfloat16
FP8 = mybir.dt.float8e4
AF = mybir.ActivationFunctionType
ALU = mybir.AluOpType
DR = mybir.MatmulPerfMode.DoubleRow
USE_DR = os.environ.get("K_USE_DR", "1") == "1"  # fp8 DoubleRow scores
DEBUG_LABELS = []  # (tile_name, fidx, pair, which) for analysis tooling


def _split_ctrl_waits(nc):
    """This walrus build only accepts a single sync-wait per instruction
    (setupSyncWait raises "Too many sync wait commands"), while Tile's
    scheduler aggregates several.  Move the excess waits onto NoOps inserted
    just before (same engine; engines execute in order, so waiting earlier
    on the same queue is equivalent)."""
    for f in nc.m.functions:
        for blk in f.blocks:
            new_list, changed = [], False
            for inst in blk.instructions:
                si = inst.sync_info
                if si is not None and len(si.on_wait) > 1:
                    waits = list(si.on_wait)
                    for w_i, w in enumerate(waits[:-1]):
                        new_list.append(
                            mybir.InstNoOp(
                                name=f"{inst.name}-waitsplit{w_i}",
                                ins=[],
                                outs=[],
                                engine=inst.engine,
                                bass_nofuse=True,
                                sync_info=mybir.SyncInfo(on_wait=[w], on_update=[]),
                            )
                        )
                    inst.sync_info = mybir.SyncInfo(
                        on_wait=[waits[-1]], on_update=list(si.on_update)
                    )
                    changed = True
                new_list.append(inst)
            if changed:
                blk.instructions = new_list


def build_nc():
    nc = bass.Bass("TRN2", target_bir_lowering=False, debug=False, num_devices=NCORES)

    # host pre-arranges inputs into the on-chip tile layout (partition-major)
    # so every load is a dense contiguous DMA.  F32R dram dtypes (same bits as
    # f32) let the loads go on any DGE queue (no gpsimd cast requirement).
    xT = nc.declare_dram_parameter("xT", [FPC, 4, 128, N], F32R, isOutput=False)
    # W1: [din-part 128, kt 4, 1536]; columns host-reordered as
    # [q0,k0,q1,k1,q2,k2,q3,k3 | v] so pair-0's weights are one tiny DMA
    # (q cols host-scaled by hd^-0.5)
    W1d = nc.declare_dram_parameter("W1d", [128, 4, 3 * D], F32R, isOutput=False)
    # W2: proj lhsT [din-part 128, kt 4, dout 512] in bf16
    W2d = nc.declare_dram_parameter("W2d", [128, 4, VD], mybir.dt.bfloat16,
                                    isOutput=False)
    bproj = nc.declare_dram_parameter("bproj", [VD], FP32, isOutput=False)
    identd = nc.declare_dram_parameter("identd", [128, 128], mybir.dt.bfloat16,
                                       isOutput=False)
    yT = nc.declare_dram_parameter("yT", [FPC, VD, N], FP32, isOutput=True)

    with tile.TileContext(nc) as tc:
        with (
            nc.allow_low_precision(
                reason="fp8e4 scores / bf16 attnv+proj matmuls (rel err ~1e-2)"
            ),
            tc.tile_pool(name="w", bufs=1) as w_pool,
            tc.tile_pool(name="x", bufs=8) as x_pool,
            tc.tile_pool(name="qk", bufs=8) as qk_pool,
            tc.tile_pool(name="v", bufs=14) as v_pool,
            tc.tile_pool(name="e", bufs=14) as e_pool,
            tc.tile_pool(name="r", bufs=2) as r_pool,
            tc.tile_pool(name="ao", bufs=8) as ao_pool,
            tc.tile_pool(name="pj", bufs=8) as pj_pool,
            tc.tile_pool(name="y", bufs=3) as y_pool,
            tc.tile_pool(name="sc", bufs=2, space="PSUM") as sc_pool,
            tc.tile_pool(name="acc", bufs=1, space="PSUM") as acc_pool,
            tc.tile_pool(name="mm", bufs=3, space="PSUM") as mm_pool,
        ):
            # ---- constants / weights (once per core) ----
            # x loads ride the gpsimd/SWDGE queue; weights ride the sync/HWDGE
            # queue in need-order (pair0-qk, v, pairs1-3, W2) so the first
            # scores matmul can start ~6us in
            W1 = w_pool.tile([128, 4, 3 * D], F32R)
            nc.sync.dma_start(out=W1[:, :, 0:256], in_=W1d[:, :, 0:256])
            W2 = w_pool.tile([128, 4, VD], BF16)
            bias_sb = w_pool.tile([128, 4], FP32)
            # constants: K=1 matmul operands for PSUM-bank zeroing + PE warmup
            onerow = w_pool.tile([1, 128], BF16)
            nc.vector.memset(onerow, 1.0)
            zrow = w_pool.tile([1, 512], BF16)
            nc.vector.memset(zrow, 0.0)
            # dummy exp: pulls the one-time ACT table load (~2.7us on HW) into
            # the initial DMA wait instead of the first scores tile
            warm_in = w_pool.tile([1, 8], FP32)
            nc.vector.memset(warm_in, 0.0)
            warm = w_pool.tile([1, 8], FP32)
            nc.scalar.activation(out=warm, in_=warm_in, func=AF.Exp)

            # q/k fp8 tiles: [128 (2 heads), 2 k-tiles, 784]; zero the second
            # k-tile half of every pool buffer once (pool rotation is
            # round-robin, so these physical halves stay zero forever).
            # Split across DVE/Pool so the early-needed buffers don't queue
            # behind the rest.
            qk_bufs = []
            if USE_DR:
                for i in range(8):
                    t = qk_pool.tile([128, 2, N], FP8, tag="qk")
                    (nc.vector if i < 4 else nc.gpsimd).memset(t[:, 1, :], 0.0)
                    qk_bufs.append(t)

            reps = int(os.environ.get("KERNEL_TIME_REPS", "1"))
            frames = [fr for _ in range(reps) for fr in range(FPC)]

            # ---- per-frame tile state ----
            X = {}       # fidx -> [4 x tiles]
            VT = {}      # fidx -> [7 v_tok tiles]
            QK = {}      # fidx -> {(p, 'q'/'k'): fp8 tile}
            AO = {}      # fidx -> attnout tile [128, 7, 512] bf16
            PJ = {}      # fidx -> [4 projin tiles [128, 7, 128] bf16]

            def load_x(fidx):
                # sync/HWDGE queue: DMA_ENGINES is a single FIFO resource, so
                # issuing everything from one queue keeps transfers in program
                # order (weights would otherwise cut ahead of x tiles)
                fr = frames[fidx]
                X[fidx] = []
                for kt in range(4):
                    xt = x_pool.tile([128, N], F32R, tag="X")
                    nc.sync.dma_start(out=xt, in_=xT[fr, kt])
                    X[fidx].append(xt)

            # ---- fill items (QKV of next frame / proj of previous) ----
            # Fill items are split into <=~450ns PE chunks: Tile's engine-tick
            # semaphores make every attention instruction transitively wait on
            # ALL earlier PE instructions, so a >1us fill right before a head
            # boundary stalls the next head's exp.
            def items_qk_mm(fidx, p, which):
                ot = 2 * p if which == "q" else 2 * p + 1
                st = {}

                def a(kts, chunk, done):
                    c0, cw = (0, 512) if chunk == "m" else (512, 272)
                    ps = st.get(chunk)
                    if ps is None:
                        if "qf" not in st:
                            if USE_DR:
                                qf = qk_pool.tile([128, 2, N], FP8, tag="qk",
                                                  name="qf")
                            else:
                                qf = qk_pool.tile([128, N], F32R, tag="qk",
                                                  name="qf")
                            DEBUG_LABELS.append((qf.tensor.name, fidx, p, which))
                            QK[fidx][(p, which)] = qf
                            st["qf"] = qf
                        ps = st[chunk] = mm_pool.tile([128, cw], FP32, tag="mm",
                                                      name="ps_qk")
                    for kt in kts:
                        nc.tensor.matmul(
                            ps,
                            W1[:, kt, ot * 128 : (ot + 1) * 128],
                            X[fidx][kt][:, c0 : c0 + cw],
                            start=(kt == 0),
                            stop=(kt == 3),
                        )
                    if done:
                        dst = (st["qf"][:, 0, c0 : c0 + cw] if USE_DR
                               else st["qf"][:, c0 : c0 + cw])
                        nc.vector.tensor_copy(dst, ps)

                return [
                    lambda: a((0, 1), "m", False),
                    lambda: a((2, 3), "m", True),
                    lambda: a((0, 1, 2, 3), "t", True),
                ]

            def items_v(fidx, jt):
                st = {}

                def a(kts, done):
                    ps = st.get("ps")
                    if ps is None:
                        ps = st["ps"] = mm_pool.tile([JT, 512], FP32, tag="mm",
                                                     name="psv")
                    for kt in kts:
                        nc.tensor.matmul(
                            ps,
                            X[fidx][kt][:, jt * JT : (jt + 1) * JT],
                            W1[:, kt, 2 * D : 3 * D],
                            start=(kt == 0),
                            stop=(kt == 3),
                        )
                    if done:
                        vt = v_pool.tile([JT, H, 65], BF16, tag="vtok",
                                         name="vt")
                        # DVE: GPSIMD cannot access PSUM on this target
                        nc.vector.tensor_copy(
                            vt[:, :, 0:HD], ps.rearrange("p (h c) -> p h c", c=HD)
                        )
                        nc.vector.memset(vt[:, :, HD : HD + 1], 1.0)
                        VT[fidx].append(vt)

                return [lambda: a((0, 1), False), lambda: a((2, 3), True)]

            def items_proj_ot(fidx, ot, kt_hi=4, st=None, kt_lo=0,
                              pool_m=None, pool_b=None):
                # chunk A: tokens 0:512 (i-tiles 0..3); B: 512:768; C: 768:784
                if st is None:
                    st = {}

                def a(chunk, kts, done):
                    key = "m" if chunk == "m" else "b"  # B and C share a bank
                    ps = st.get(key)
                    if ps is None:
                        shape = [128, 512] if chunk == "m" else [128, 272]
                        pool = (pool_m if key == "m" else pool_b) or mm_pool
                        ps = st[key] = pool.tile(shape, FP32, tag="mm",
                                                 name="ps_pj")
                        if key == "b":
                            # B and C are separate accumulation groups in one
                            # bank: zero it once, groups run start=False
                            nc.tensor.matmul(ps, onerow, zrow[:, 0:272],
                                             start=True, stop=True)
                    pj = PJ[fidx]
                    for kt in kts:
                        w = W2[:, kt, ot * 128 : (ot + 1) * 128]
                        if chunk == "m":
                            nc.tensor.matmul(ps, w, pj[kt][:, 0:4, :],
                                             start=(kt == 0), stop=(kt == 3))
                        elif chunk == "b":
                            nc.tensor.matmul(ps[:, 0:256], w, pj[kt][:, 4:6, :],
                                             start=False, stop=(kt == 3),
                                             skip_group_check=True)
                        else:
                            nc.tensor.matmul(ps[:, 256:272], w,
                                             pj[kt][:, 6, 112:128],
                                             start=False, stop=(kt == 3),
                                             skip_group_check=True)
                    if done:
                        yt = y_pool.tile([128, N], FP32, tag="yT", name="yt")
                        # DVE: GPSIMD cannot access PSUM; ACT is the
                        # bottleneck engine so bias stays off it
                        nc.vector.tensor_scalar_add(
                            yt[:, 0:512], st["m"], bias_sb[:, ot : ot + 1]
                        )
                        nc.vector.tensor_scalar_add(
                            yt[:, 512:784], st["b"], bias_sb[:, ot : ot + 1]
                        )
                        fr = frames[fidx]
                        # split store: the main half leaves while the tail
                        # bias still runs, shortening the final-drain chain
                        nc.sync.dma_start(
                            out=yT[fr, ot * 128 : (ot + 1) * 128, 0:512],
                            in_=yt[:, 0:512],
                        )
                        nc.sync.dma_start(
                            out=yT[fr, ot * 128 : (ot + 1) * 128, 512:784],
                            in_=yt[:, 512:784],
                        )

                kts = tuple(range(kt_lo, kt_hi))
                out = [
                    lambda g=kts[i : i + 2]: a("m", g, False)
                    for i in range(0, len(kts), 2)
                ]
                out.append(lambda: a("b", kts, False))
                out.append(lambda: a("c", kts, kt_hi == 4))
                return out

            # ---- attention phase building blocks ----
            def emit_scores(fidx, h, jt, sc_t):
                p, hh = h // 2, h % 2
                qf = QK[fidx][(p, "q")]
                kf = QK[fidx][(p, "k")]
                j0 = jt * JT
                for c0, cw in CHUNKS:
                    if USE_DR:
                        nc.tensor.matmul(
                            sc_t[0:JT, c0 : c0 + cw],
                            kf[64 * hh : 64 * hh + 64, :, j0 : j0 + JT],
                            qf[64 * hh : 64 * hh + 64, :, c0 : c0 + cw],
                            start=True,
                            stop=True,
                            perf_mode=DR,
                        )
                    else:
                        nc.tensor.matmul(
                            sc_t[0:JT, c0 : c0 + cw],
                            kf[64 * hh : 64 * hh + 64, j0 : j0 + JT],
                            qf[64 * hh : 64 * hh + 64, c0 : c0 + cw],
                            start=True,
                            stop=True,
                            tile_position=(64 * hh, 0),
                        )

            def emit_exp(h, jt, sc_t):
                e_t = e_pool.tile([JT, N], BF16, tag="expT")
                nc.scalar.activation(out=e_t, in_=sc_t[0:JT, 0:784], func=AF.Exp)
                return e_t

            def emit_attnv_jt(fidx, h, jt, e_t, acc):
                # all matmuls accumulate start=False onto the pre-zeroed bank
                # (RAW dep on the zeroing matmul orders them; scheduler-safe)
                vt = VT[fidx][jt]
                for k, io in enumerate(IT0):
                    nc.tensor.matmul(
                        acc[:, 65 * k : 65 * k + 65],
                        e_t[:, io : io + 128],
                        vt[:, h, :],
                        start=False,
                        stop=(jt == NJ - 1),
                        skip_group_check=True,
                    )

            def emit_normalize(fidx, h, acc):
                r_sb = r_pool.tile([128, 8], FP32, tag="rsb")
                accv = acc[:, 0:455].rearrange("p (k c) -> p k c", c=65)
                if os.environ.get("K_SAFE_NORM", "0") == "1":
                    # conservative forms: contiguous-SBUF reciprocal input and
                    # per-partition-scalar multiplies
                    s_sb = r_pool.tile([128, 8], FP32, tag="ssb")
                    nc.vector.tensor_copy(
                        s_sb[:, 0:7].unsqueeze(2), accv[:, :, 64:65]
                    )
                    nc.vector.reciprocal(out=r_sb[:, 0:7], in_=s_sb[:, 0:7])
                    for k in range(NJ):
                        nc.vector.tensor_scalar_mul(
                            AO[fidx][h // 2][:, k, HD * (h % 2) : HD * (h % 2 + 1)],
                            accv[:, k, 0:HD],
                            r_sb[:, k : k + 1],
                        )
                else:
                    nc.vector.reciprocal(
                        out=r_sb[:, 0:7].unsqueeze(2), in_=accv[:, :, 64:65]
                    )
                    nc.vector.tensor_tensor(
                        out=AO[fidx][h // 2][:, :, HD * (h % 2) : HD * (h % 2 + 1)],
                        in0=accv[:, :, 0:HD],
                        in1=r_sb[:, 0:7].unsqueeze(2).broadcast_to([128, 7, HD]),
                        op=ALU.mult,
                    )

            def emit_transposes(fidx, b):
                # attnout of pair b -> channel-major projin via plain matmul
                # against the identity: out[d, i] = sum_i' AO[i', d] I[i', i].
                # (The DMA XBAR transpose faults the core on this runtime.)
                pjb = pj_pool.tile([128, 7, 128], BF16, tag="pj")
                PJ[fidx].append(pjb)
                for k in range(NJ):
                    pst = mm_pool.tile([128, 128], FP32, tag="mm", name="pst")
                    nc.tensor.matmul(
                        pst, AO[fidx][b][:, k, :], ident, start=True, stop=True
                    )
                    nc.vector.tensor_copy(pjb[:, k, :], pst)

            # ---- preamble: frame-0 x + remaining weights, single queue in
            # need-order (pair0-qk done above, x0, v-weights, rest); warmup ----
            load_x(0)
            nc.sync.dma_start(out=W1[:, :, 2 * D : 3 * D], in_=W1d[:, :, 2 * D :])
            for p in range(1, 4):
                nc.sync.dma_start(
                    out=W1[:, :, 256 * p : 256 * (p + 1)],
                    in_=W1d[:, :, 256 * p : 256 * (p + 1)],
                )
            nc.sync.dma_start(out=W2, in_=W2d[:])
            nc.sync.dma_start(out=bias_sb, in_=bproj.rearrange("(a p) -> p a", p=128))
            ident = w_pool.tile([128, 128], BF16)
            nc.sync.dma_start(out=ident, in_=identd[:])
            # PE p-state warmup: dependency-free matmuls run during the DMA
            # wait so pe_busy_start predates the first real matmul
            wacc = acc_pool.tile([128, 512], FP32, tag="oacc")
            for _ in range(10):
                nc.tensor.matmul(wacc, onerow, zrow, start=True, stop=True)

            # ---- frame 0: only pair-0 q/k up front; V + remaining pairs go
            # through the fill queue like every other frame ----
            QK[0] = {}
            VT[0] = []
            for wq in ("q", "k"):
                for it in items_qk_mm(0, 0, wq):
                    it()

            fills = []

            def add_fills(items, min_slot=0, pos=None):
                entries = [(min_slot, it) for it in items]
                if pos is None:
                    fills.extend(entries)
                else:
                    fills[pos:pos] = entries

            for j in range(NJ):
                add_fills(items_v(0, j))
            for p in range(1, 4):
                # pair p's W1 qk columns land ~(10 + 1.6p)us (~slot 8 + 2p)
                add_fills(items_qk_mm(0, p, "q") + items_qk_mm(0, p, "k"),
                          min_slot=10 + 4 * p)
            slot = [0]

            def pump(force=False):
                # one micro-item (<=~450ns of PE work) per pump slot fits the
                # ~480ns PE slack per 838ns exp period; entries carry a
                # minimum slot so sparse queues spread instead of bunching.
                # force (frame-0 backlog) drains two per slot.
                slot[0] += 1
                for _ in range(2 if force else 1):
                    if fills and slot[0] >= fills[0][0]:
                        fills.pop(0)[1]()

            for fidx in range(len(frames)):
                has_next = fidx + 1 < len(frames)
                AO[fidx] = [
                    ao_pool.tile([128, NJ, 128], BF16, tag="ao", name="attnout")
                    for _ in range(4)
                ]
                PJ[fidx] = []
                slot[0] = 0
                # leftovers from the previous frame are dep-ready now
                fills[:] = [(0, fn) for _, fn in fills]

                if fidx - 1 >= 0:
                    # previous frame's projection first.  d-blocks 0-2 were
                    # transposed mid-frame; only the kt=3 finisher must wait
                    # for the pair-3 transposes (~5us in, ~slot 7)
                    prev = fidx - 1
                    k = 0
                    stride = 1 if has_next else 2
                    for ot in range(4):
                        st = {}
                        for it in items_proj_ot(prev, ot, kt_hi=3, st=st):
                            fills.append((1 + stride * k, it))
                            k += 1
                        for it in items_proj_ot(prev, ot, kt_lo=3, st=st):
                            # pair-3 transposes of the previous frame only
                            # complete ~slot 12 (they start after its h7
                            # normalize, which overlaps this frame's h0-h1)
                            fills.append((max(18, 1 + stride * k), it))
                            k += 1
                if has_next:
                    # next frame's QKV, gated on when its deps resolve: x DMAs
                    # land ~slot 8; pair p's q/k tiles are WAR-blocked until
                    # this frame's pair-p scores finish (~slot 16p+18)
                    load_x(fidx + 1)
                    QK[fidx + 1] = {}
                    VT[fidx + 1] = []
                    nxt = fidx + 1
                    for j in range(NJ):
                        add_fills(items_v(nxt, j), min_slot=10)
                    for p in range(4):
                        add_fills(
                            items_qk_mm(nxt, p, "q") + items_qk_mm(nxt, p, "k"),
                            min_slot=26 * p + 24,
                        )

                for h in range(H):
                    acc = acc_pool.tile([128, 512], FP32, tag="oacc")
                    # frame-0 head-0 is forced so its V fills land before
                    # head 0's attn@v consumes them
                    force = fidx == 0 and h == 0
                    # single start=True zeroing matmul owns the bank's 2KB
                    # zero-region; every attn@v matmul accumulates start=False
                    # with a RAW dep on it, which also stops the scheduler
                    # from interleaving accumulation-group starts
                    nc.tensor.matmul(acc[:, 0:512], onerow, zrow,
                                     start=True, stop=True)
                    pend = []
                    for jt in range(NJ):
                        sc_t = sc_pool.tile([128, 1024], FP32, tag="sc")
                        emit_scores(fidx, h, jt, sc_t)
                        e_t = emit_exp(h, jt, sc_t)
                        pend.append((jt, e_t))
                        if len(pend) > PIPE:
                            jt_, e_ = pend.pop(0)
                            if force:
                                while len(VT[fidx]) <= jt_ and fills:
                                    fills.pop(0)[1]()
                            pump(force=force)
                            emit_attnv_jt(fidx, h, jt_, e_, acc)
                    for jt_, e_ in pend:
                        if force:
                            while len(VT[fidx]) <= jt_ and fills:
                                fills.pop(0)[1]()
                        pump(force=force)
                        emit_attnv_jt(fidx, h, jt_, e_, acc)
                    emit_normalize(fidx, h, acc)
                    pump()
                    if h % 2 == 1:
                        emit_transposes(fidx, h // 2)
                        pump()
                        if not has_next and h == 5:
                            # prestage the last frame's first proj block over
                            # d-blocks 0..2 (their transposes are done) so the
                            # drain only owes kt=3 + the remaining blocks
                            st0 = {}
                            add_fills(items_proj_ot(fidx, 0, kt_hi=3, st=st0),
                                      pos=0)
                            PJ.setdefault(("st0", fidx), st0)
                        if not has_next and h == 7:
                            # ot1 prestages into a retired scores tile (its
                            # last reader is this head's exp stream)
                            sct = sc_pool.tile([128, 1024], FP32, tag="sc",
                                               name="drain_sc")
                            st1 = {"m": sct[:, 0:512], "b": sct[:, 512:784]}
                            nc.tensor.matmul(st1["b"], onerow, zrow[:, 0:272],
                                             start=True, stop=True)
                            add_fills(items_proj_ot(fidx, 1, kt_hi=3, st=st1),
                                      pos=0)
                            PJ.setdefault(("st1", fidx), st1)

                if not has_next:
                    # drain: flush any unpumped fill items first (the h7
                    # prestage lands after the frame's last pump slot)
                    while fills:
                        fills.pop(0)[1]()
                    # ot2 prestages into the second retired scores
                    # tile; dummies keep the PE at full P-state across the
                    # last transpose's ~3.5us DMA latency; then the kt=3
                    # finishers and ot3 (whose mm tiles ot0's finisher frees)
                    sct2 = sc_pool.tile([128, 1024], FP32, tag="sc",
                                        name="drain_sc2")
                    st2 = {"m": sct2[:, 0:512], "b": sct2[:, 512:784]}
                    nc.tensor.matmul(st2["b"], onerow, zrow[:, 0:272],
                                     start=True, stop=True)
                    for it in items_proj_ot(fidx, 2, kt_hi=3, st=st2):
                        it()
                    st0 = PJ.pop(("st0", fidx), {})
                    st1 = PJ.pop(("st1", fidx), {})
                    for st in (st0, st1, st2):
                        for it in items_proj_ot(fidx, (st0, st1, st2).index(st),
                                                kt_lo=3, st=st):
                            it()
                    for it in items_proj_ot(fidx, 3):
                        it()
                # leftover fills roll into the next frame's slots

                # drop per-frame tile references we no longer need
                if fidx - 1 >= 0:
                    for d in (X, VT, QK, AO, PJ):
                        d.pop(fidx - 1, None)

    _split_ctrl_waits(nc)
    return nc


_CACHE = {}


def _get_runner():
    """Build the Bass module once and wrap it in a cached sharded jax.jit
    callable (replicates bass2jax.run_bass_via_pjrt but reusable across
    calls, so repeated invocations don't re-lower/re-compile)."""
    if "runner" in _CACHE:
        return _CACHE["runner"]

    import jax
    from jax.experimental.shard_map import shard_map
    from jax.sharding import Mesh, PartitionSpec
    from concourse import bass2jax, mybir as _mybir

    nc = build_nc()
    bass2jax.install_neuronx_cc_hook()
    assert nc.dbg_addr is None
    partition_name = nc.partition_id_tensor.name if nc.partition_id_tensor else None

    in_names, out_names, out_avals, out_shapes = [], [], [], []
    for alloc in nc.m.functions[0].allocations:
        if not isinstance(alloc, _mybir.MemoryLocationSet):
            continue
        name = alloc.memorylocations[0].name
        if alloc.kind == "ExternalInput":
            if name != partition_name:
                in_names.append(name)
        elif alloc.kind == "ExternalOutput":
            shape = tuple(alloc.tensor_shape)
            dtype = _mybir.dt.np(alloc.dtype)
            out_names.append(name)
            out_avals.append(jax.core.ShapedArray(shape, dtype))
            out_shapes.append((shape, dtype))
    n_params = len(in_names)
    all_names = in_names + out_names
    if partition_name is not None:
        all_names = all_names + [partition_name]

    def _body(*args):
        operands = list(args)
        if partition_name is not None:
            operands.append(bass2jax.partition_id_tensor())
        outs = bass2jax._bass_exec_p.bind(
            *operands,
            out_avals=tuple(out_avals),
            in_names=tuple(all_names),
            out_names=tuple(out_names),
            lowering_input_output_aliases=(),
            sim_require_finite=True,
            sim_require_nnan=True,
            nc=nc,
        )
        return tuple(outs)

    devices = jax.devices()[:NCORES]
    mesh = Mesh(np.asarray(devices), ("core",))
    nin = n_params + len(out_names)
    sharded = jax.jit(
        shard_map(
            _body,
            mesh=mesh,
            in_specs=(PartitionSpec("core"),) * nin,
            out_specs=(PartitionSpec("core"),) * len(out_names),
            check_rep=False,
        ),
        donate_argnums=tuple(range(n_params, nin)),
        keep_unused=True,
    )

    def run(in_maps):
        concat_in = [
            np.concatenate([np.asarray(m[name]) for m in in_maps], axis=0)
            for name in in_names
        ]
        last_err = None
        for attempt in range(3):
            # fresh zeros each attempt (donated buffers are consumed even on
            # a failed dispatch)
            concat_zeros = [
                np.zeros((NCORES * s[0], *s[1:]), dt) for s, dt in out_shapes
            ]
            try:
                out_arrs = sharded(*concat_in, *concat_zeros)
                # materialize inside the retry scope: transient device errors
                # (e.g. NRT_EXEC_UNIT_UNRECOVERABLE through the axon relay)
                # can surface at fetch time
                host = [np.asarray(o) for o in out_arrs]
                return [
                    {
                        name: host[i].reshape(NCORES, *out_shapes[i][0])[c]
                        for i, name in enumerate(out_names)
                    }
                    for c in range(NCORES)
                ]
            except Exception as e:  # noqa: BLE001 - retry transient device faults
                last_err = e
                import time as _time

                _time.sleep(2.0 * (attempt + 1))
        raise last_err

    _CACHE["runner"] = run
    _CACHE["parts"] = dict(
        nc=nc, sharded=sharded, in_names=in_names, out_names=out_names,
        out_shapes=out_shapes, mesh=mesh, n_params=n_params,
    )
    return run


def prepare_in_maps(x, Wqkv, Wproj, bproj):
    import ml_dtypes

    x = np.ascontiguousarray(np.asarray(x, dtype=np.float32))
    Wqkv = np.array(np.asarray(Wqkv, dtype=np.float32))
    Wproj = np.asarray(Wproj, dtype=np.float32)
    bp = np.ascontiguousarray(np.asarray(bproj, dtype=np.float32))

    # fold the attention scale into Wq so neither exp nor the fp8 cast needs it
    Wqkv[0:D, :] = Wqkv[0:D, :] * (HD ** -0.5)
    # reorder qk output channels pairwise: [q0,k0,q1,k1,q2,k2,q3,k3 | v]
    order = []
    for p in range(4):
        order += list(range(128 * p, 128 * p + 128))          # q pair p
        order += list(range(D + 128 * p, D + 128 * p + 128))  # k pair p
    order += list(range(2 * D, 3 * D))                        # v
    Wqkv = Wqkv[order, :]

    # (b, f*n, d) -> (b*f, kt, p, n) channel-major tiles
    xt = np.ascontiguousarray(
        x.reshape(B * F, N, 4, 128).transpose(0, 2, 3, 1)
    )
    # [d_in, m] -> [p, kt, m] partition-major tiles
    W1d = np.ascontiguousarray(
        Wqkv.T.reshape(4, 128, 3 * D).transpose(1, 0, 2)
    )
    W2d = np.ascontiguousarray(
        Wproj.T.reshape(4, 128, VD).transpose(1, 0, 2)
    ).astype(ml_dtypes.bfloat16)
    ident = np.eye(128, dtype=ml_dtypes.bfloat16)
    return [
        {
            "xT": np.ascontiguousarray(xt[c * FPC : (c + 1) * FPC]),
            "W1d": W1d,
            "W2d": W2d,
            "bproj": bp,
            "identd": ident,
        }
        for c in range(NCORES)
    ]


def kernel(x, Wqkv, Wproj, bproj, spatial=None, f=None, n=None, **_ignored):
    in_maps = prepare_in_maps(x, Wqkv, Wproj, bproj)
    results = _get_runner()(in_maps)

    y = np.empty((B * F, N, VD), dtype=np.float32)
    for c in range(NCORES):
        y[c * FPC : (c + 1) * FPC] = results[c]["yT"].transpose(0, 2, 1)
    return y.reshape(B, F * N, VD)
